# revision 26
# baseline (speedup 1.0000x reference)
"""DiffPool GNN forward on 8 Trainium2 NeuronCores.

Data-parallel over the batch dim (B=16 -> 2 batches per core). Host packs
per-batch dense transposed features (bf16) and a column-trimmed transposed
dense adjacency (bf16); each core runs the DiffPool batched GEMMs locally.

Structure per core (emission order = per-engine execution order):
  warmup MMs (HAM un-throttle) -> proj b0, proj b1 (bf16, fused pool|emb
  weights) -> b0 t-GEMM pass A (6 PSUM banks, v-outer, consumes adjacency
  slabs as they stream) + pass B (u-outer) -> a1t/x1t b0 -> b1 passes with
  the level-2 stages of b0 interleaved between v/u-groups -> level-2 b1.

Level-2 is restructured transpose-free: every product is emitted in the
orientation whose result is directly the lhsT of its consumer, so no
PE-transpose / copy pairs are needed. Level-2 stays f32 (softmax of s2 is
sensitive); level-1 operands are bf16.
"""

import numpy as np
import ml_dtypes

import concourse.bass as bass
import concourse.mybir as mybir
from concourse import tile
from concourse.bass_utils import run_bass_kernel_spmd

# ---------------------------------------------------------------------------
# Problem constants (hardcoded per spec; setup_inputs has n1=1100, n2=900)
# ---------------------------------------------------------------------------
B = 16
NCORES = 8
BPC = B // NCORES          # batches per core
MN = 2048                  # MAX_NODES
IN_DIM = 128
HID = 64
OUT = 2
K1 = 205
K2 = 21
N1P = 1100                 # g1 nodes per batch (constant in generator)
N2P = 900
WTRIM = 1152               # trimmed slab width: cols (src) kept for v-slabs 0..7
NPROJ = K1 + HID           # fused pool|emb projection width

F32 = mybir.dt.float32
BF16 = mybir.dt.bfloat16
F8 = mybir.dt.float8e4
AF = mybir.ActivationFunctionType
BFNP = ml_dtypes.bfloat16
F8NP = ml_dtypes.float8_e4m3

# adjacency dtype: fp8 halves HBM traffic; entries are exactly 0/1 so the
# only risk is tensor-engine support for fp8 weights x bf16 moving operand
ADJ_FP8 = True
ADJ_DT = F8 if ADJ_FP8 else BF16
ADJ_ONE = 0x38 if ADJ_FP8 else 0x3F80
ADJ_NP = F8NP if ADJ_FP8 else BFNP

# packed adjacency layout: per batch [128, TOTC]; partition = dst&127,
# column block v = dst>>7 at OFFV[v], col within block = src (trimmed to
# 1152 for v<8 where dst is a g1 node so src < 1152)
WIDV = [WTRIM] * 8 + [MN] * 8
OFFV = [0] * 16
for _v in range(1, 16):
    OFFV[_v] = OFFV[_v - 1] + WIDV[_v - 1]
TOTC = OFFV[15] + WIDV[15]             # 25600
XTC = WTRIM + 1024                     # packed features width 2176

_M2 = ((0, 128), (128, K1 - 128))      # row tiling of 205-row matrices

# level-2 weight pack layout: [64, 472] f32
_W2COLS = {
    "Wp1": (64, 0, 21), "Up1": (64, 21, 42), "Wp2": (21, 42, 63),
    "Up2": (21, 63, 84), "We1": (64, 84, 148), "Ue1": (64, 148, 212),
    "We2": (64, 212, 276), "Ue2": (64, 276, 340), "Wc1": (64, 340, 404),
    "Uc1": (64, 404, 468), "Wc2": (64, 468, 470), "Uc2": (64, 470, 472),
}
W2W = 472


# ---------------------------------------------------------------------------
# Walrus workaround: this toolchain's walrus encodes at most ONE sync wait
# per instruction; split multi-wait instructions via single-wait NOPs.
# ---------------------------------------------------------------------------
_mw_ctr = [0]


def _legalize_multiwait(nc):
    for func in nc.m.functions:
        for bb in func.blocks:
            insts = bb.instructions
            new = []
            changed = False
            for ins in insts:
                si = getattr(ins, "sync_info", None)
                waits = list(si.on_wait) if (si and si.on_wait) else []
                if len(waits) > 1:
                    changed = True
                    for w in waits[:-1]:
                        _mw_ctr[0] += 1
                        nop = mybir.InstNoOp(
                            name=f"mwfix-{_mw_ctr[0]}",
                            engine=ins.engine,
                            ins=[],
                            outs=[],
                            sync_info=mybir.SyncInfo(on_wait=[w], on_update=[]),
                            bass_nofuse=True,
                        )
                        nc.register_instruction(nop, overwrite=True)
                        new.append(nop)
                    si.on_wait = [waits[-1]]
                new.append(ins)
            if changed:
                bb.instructions[:] = new


# ---------------------------------------------------------------------------
# Device program
# ---------------------------------------------------------------------------
def build_nc(debug=False):
    nc = bass.Bass()

    xt = nc.dram_tensor("xt", [BPC, IN_DIM, XTC], BF16, kind="ExternalInput")
    adj = nc.dram_tensor("adj", [BPC, 128, TOTC], ADJ_DT,
                         kind="ExternalInput")
    wall = nc.dram_tensor("wall", [IN_DIM, 2 * NPROJ], BF16,
                          kind="ExternalInput")
    w2 = nc.dram_tensor("w2", [HID, W2W], F32, kind="ExternalInput")
    out = nc.dram_tensor("out", [BPC, OUT], F32, kind="ExternalOutput")
    if debug:
        dbg = {}
        for nm, shp in [("s", [MN, K1]), ("t", [MN, K1]), ("a1t", [K1, K1]),
                        ("x1t", [HID, K1]), ("sm2", [K1, K2]),
                        ("x1e", [K1, HID]), ("x2t", [HID, K2]),
                        ("a2t", [K2, K2])]:
            dbg[nm] = nc.dram_tensor(f"dbg_{nm}", shp, F32,
                                     kind="ExternalOutput")

    with tile.TileContext(nc) as tc:
        with (
            tc.tile_pool(name="const", bufs=1) as cpool,
            tc.tile_pool(name="xtp", bufs=2) as xtp,
            tc.tile_pool(name="slab", bufs=2) as slabp,
            tc.tile_pool(name="act", bufs=1) as actp,
            tc.tile_pool(name="tt", bufs=2) as tp,
            tc.tile_pool(name="l2", bufs=2) as l2p,
            tc.tile_pool(name="smx", bufs=3) as smxp,
            tc.tile_pool(name="psA", bufs=6, space="PSUM") as psA,
            tc.tile_pool(name="psS", bufs=2, space="PSUM") as psS,
        ):
            # ---- constants / weights ----
            out_sbs = [cpool.tile([1, OUT], F32, tag=f"out_sb{b}",
                                  name="osb") for b in range(BPC)]

            # feature/weight DMAs go on the scalar engine's queue so they run
            # concurrently with the adjacency stream on the sync queue
            wall_sb = cpool.tile([IN_DIM, 2 * NPROJ], BF16, tag="wall")
            nc.scalar.dma_start(out=wall_sb[:], in_=wall[:])
            w2_sb = cpool.tile([HID, W2W], F32, tag="w2")
            nc.scalar.dma_start(out=w2_sb[:], in_=w2[:])

            # warmup: PE busy from t~8us so the HAM clock gate opens before
            # the projections start (it needs ~3.4us of sustained activity)
            warm = cpool.tile([128, 256], BF16, tag="warm")
            nc.gpsimd.memset(warm[:], 0.0)
            for _ in range(6):
                pw = psS.tile([128, 256], F32, tag="mm", name="pw")
                nc.tensor.matmul(pw[:], lhsT=warm[:, :128], rhs=warm[:],
                                 start=True, stop=True)

            def w2ap(name):
                rows, c0, c1 = _W2COLS[name]
                return w2_sb[:rows, c0:c1]

            # ---- per-batch state ----
            s_bf = [[None] * 16, [None] * 16]
            h_bf = [[None] * 16, [None] * 16]
            adj_sb = [None, None]
            t_bf = [[None] * 16, [None] * 16]
            a1t = [[None, None], [None, None]]
            x1t = [None, None]

            # ---- projections: s = softmax(x@Wpool), h = relu(x@Wemb) ----
            # both feature DMAs issued upfront (scalar queue) so the second
            # batch's transfer isn't queued behind batch 0's activation ops
            xsbs = []
            for b in range(BPC):
                xsb = xtp.tile([IN_DIM, XTC], BF16, tag="xt", name="xsb")
                nc.scalar.dma_start(out=xsb[:], in_=xt[b])
                xsbs.append(xsb)

            def proj(b):
                xsb = xsbs[b]
                for i in range(16):
                    pp = psS.tile([128, NPROJ], F32, tag="mm", name="pp")
                    if i < 8:
                        nc.tensor.matmul(pp[:], lhsT=xsb[:, i * 128:(i + 1) * 128],
                                         rhs=wall_sb[:, :NPROJ],
                                         start=True, stop=True)
                    elif i == 8:
                        nc.tensor.matmul(pp[:], lhsT=xsb[:, 1024:WTRIM],
                                         rhs=wall_sb[:, :NPROJ],
                                         start=True, stop=False)
                        nc.tensor.matmul(pp[:], lhsT=xsb[:, WTRIM:WTRIM + 128],
                                         rhs=wall_sb[:, NPROJ:],
                                         start=False, stop=True)
                    else:
                        c0 = 128 * (i + 1)
                        nc.tensor.matmul(pp[:], lhsT=xsb[:, c0:c0 + 128],
                                         rhs=wall_sb[:, NPROJ:],
                                         start=True, stop=True)
                    # softmax without max-subtract: |scores| < 0.2 by scale
                    st = actp.tile([128, K1], BF16, tag=f"s{b}_{i}", name="st")
                    ssum = smxp.tile([128, 1], F32, tag="ssum", name="ssum")
                    nc.scalar.activation(out=st[:], in_=pp[:, :K1],
                                         func=AF.Exp, scale=1.0,
                                         accum_out=ssum[:])
                    rinv = smxp.tile([128, 1], F32, tag="rinv", name="rinv")
                    nc.vector.reciprocal(out=rinv[:], in_=ssum[:])
                    nc.vector.tensor_scalar_mul(out=st[:], in0=st[:],
                                                scalar1=rinv[:])
                    ht = actp.tile([128, HID], BF16, tag=f"h{b}_{i}", name="ht")
                    nc.scalar.activation(out=ht[:], in_=pp[:, K1:],
                                         func=AF.Relu)
                    s_bf[b][i] = st
                    h_bf[b][i] = ht

            # adjacency DMA: 4 column-group transfers per batch, each into its
            # own tile (distinct dep per group) so pass A starts on group 0;
            # rows are long contiguous runs (fast DMA)
            ADJ_GROUPS = [(OFFV[0], OFFV[4]), (OFFV[4], OFFV[8]),
                          (OFFV[8], OFFV[12]), (OFFV[12], TOTC)]

            def emit_adj_dmas(b):
                tiles = []
                for gi, (c0, c1) in enumerate(ADJ_GROUPS):
                    g = slabp.tile([128, c1 - c0], ADJ_DT, tag=f"adjg{gi}",
                                   name="g")
                    nc.sync.dma_start(out=g[:], in_=adj[b][:, c0:c1])
                    tiles.append(g)
                adj_sb[b] = tiles

            def adj_ap(b, v, u):
                gi = v // 4
                c0 = OFFV[v] - ADJ_GROUPS[gi][0] + u * 128
                return adj_sb[b][gi][:, c0:c0 + 128]

            # ---- t = adj @ s (u-blocked into PSUM banks; skip the zero
            #      block: u-slabs 9..15 x v-slabs 0..7 of adj are zero) ----
            NA = 6   # pass-A u-group width = psA bufs

            def cast_t(b, u, tacc):
                tt = tp.tile([128, K1], BF16, tag=f"t{u}", name="tt")
                if u % 2 == 0:
                    nc.vector.tensor_copy(out=tt[:], in_=tacc[:])
                else:
                    nc.scalar.activation(out=tt[:], in_=tacc[:], func=AF.Copy,
                                         scale=1.0)
                t_bf[b][u] = tt

            def passA_v(b, v, taccs):
                if v == 0:
                    for u in range(NA):
                        taccs.append(psA.tile([128, K1], F32, tag="tacc",
                                              name="tacc"))
                for u in range(NA):
                    nc.tensor.matmul(taccs[u][:],
                                     lhsT=adj_ap(b, v, u),
                                     rhs=s_bf[b][v][:],
                                     start=(v == 0), stop=(v == 15))
                if v == 15:
                    for u in range(NA):
                        cast_t(b, u, taccs[u])

            def passB_u(b, u):
                vs = list(range(16)) if u <= 8 else list(range(8, 16))
                tacc = psA.tile([128, K1], F32, tag="tacc", name="tacc")
                for v in vs:
                    nc.tensor.matmul(tacc[:],
                                     lhsT=adj_ap(b, v, u),
                                     rhs=s_bf[b][v][:],
                                     start=(v == vs[0]), stop=(v == vs[-1]))
                cast_t(b, u, tacc)

            # ---- a1t = t^T s  [205,205] (row-tiled), x1t = h^T s [64,205] ----
            def a1t_m(b, mi):
                m0, msz = _M2[mi]
                pa = psS.tile([128, K1], F32, tag="mm", name="pa")
                for v in range(16):
                    nc.tensor.matmul(pa[:msz, :],
                                     lhsT=t_bf[b][v][:, m0:m0 + msz],
                                     rhs=s_bf[b][v][:],
                                     start=(v == 0), stop=(v == 15))
                asb = l2p.tile([128, K1], F32, tag=f"a1t{mi}", name="asb")
                nc.vector.tensor_copy(out=asb[:msz, :], in_=pa[:msz, :])
                a1t[b][mi] = asb

            def x1t_u(b):
                px = psS.tile([HID, K1], F32, tag="mm", name="px")
                for v in range(16):
                    nc.tensor.matmul(px[:], lhsT=h_bf[b][v][:],
                                     rhs=s_bf[b][v][:],
                                     start=(v == 0), stop=(v == 15))
                xsb = l2p.tile([HID, K1], F32, tag="x1t", name="xsb")
                nc.vector.tensor_copy(out=xsb[:], in_=px[:])
                x1t[b] = xsb

            # ---- level-2: transpose-free stage list ----
            def lvl2_stages(b):
                at, xt_ = a1t[b], x1t[b]
                T = {}

                def wmm205(rhs_ap, n, tag, relu=False):
                    """out[205,n] = x1 @ W as 2 row-tiles: lhsT=x1t col-slice"""
                    outs = []
                    for mi, (m0, msz) in enumerate(_M2):
                        p = psS.tile([128, n], F32, tag="mm", name="p")
                        nc.tensor.matmul(p[:msz, :], lhsT=xt_[:, m0:m0 + msz],
                                         rhs=rhs_ap, start=True, stop=True)
                        o = l2p.tile([128, n], F32, tag=f"{tag}{mi}", name="o")
                        nc.vector.tensor_copy(out=o[:msz, :], in_=p[:msz, :])
                        outs.append(o)
                    return outs

                def hhT(z1, U1, n, tag):
                    """hhT[n,205] = relu((a1 @ z1 + x1 @ U1)^T)"""
                    p = psS.tile([n, K1], F32, tag="mm", name="p")
                    for ki, (k0, ksz) in enumerate(_M2):
                        nc.tensor.matmul(p[:], lhsT=z1[ki][:ksz, :n],
                                         rhs=at[ki][:ksz, :],
                                         start=(ki == 0), stop=False)
                    nc.tensor.matmul(p[:], lhsT=w2ap(U1)[:, :n], rhs=xt_[:],
                                     start=False, stop=True)
                    o = l2p.tile([n, K1], F32, tag=tag, name="o")
                    nc.scalar.activation(out=o[:], in_=p[:], func=AF.Relu)
                    T[tag] = o
                    return o

                def z2s_m(hh, W2n, n, tag):
                    """z2[205,n] = hh @ W2 as row-tiles: lhsT=hhT col-slice"""
                    outs = []
                    nh = hh.shape[0]
                    for mi, (m0, msz) in enumerate(_M2):
                        p = psS.tile([128, n], F32, tag="mm", name="p")
                        nc.tensor.matmul(p[:msz, :], lhsT=hh[:nh, m0:m0 + msz],
                                         rhs=w2ap(W2n)[:nh, :n],
                                         start=True, stop=True)
                        o = l2p.tile([128, n], F32, tag=f"{tag}{mi}", name="o")
                        nc.scalar.activation(out=o[:msz, :], in_=p[:msz, :],
                                             func=AF.Copy, scale=1.0)
                        outs.append(o)
                    return outs

                def stage_o(z2, hh, U2, n, tag, softmax):
                    """o[205,n] = a1 @ z2 + hh @ U2, per row-tile; optionally
                    softmax along free dim into tag tiles."""
                    outs = []
                    nh = hh.shape[0]
                    for mi, (m0, msz) in enumerate(_M2):
                        p = psS.tile([128, n], F32, tag="mm", name="p")
                        for ki, (k0, ksz) in enumerate(_M2):
                            nc.tensor.matmul(p[:msz, :],
                                             lhsT=at[ki][:ksz, m0:m0 + msz],
                                             rhs=z2[ki][:ksz, :],
                                             start=(ki == 0), stop=False)
                        nc.tensor.matmul(p[:msz, :],
                                         lhsT=hh[:nh, m0:m0 + msz],
                                         rhs=w2ap(U2)[:nh, :n],
                                         start=False, stop=True)
                        o = l2p.tile([128, n], F32, tag=f"{tag}{mi}", name="o")
                        if softmax:
                            nmax = smxp.tile([128, 1], F32, tag="nmax",
                                             name="nmax")
                            nc.vector.reduce_max(out=nmax[:msz], in_=p[:msz, :],
                                                 axis=mybir.AxisListType.X,
                                                 negate=True)
                            ssum = smxp.tile([128, 1], F32, tag="ssum",
                                             name="ssum")
                            nc.scalar.activation(out=o[:msz, :], in_=p[:msz, :],
                                                 func=AF.Exp, bias=nmax[:msz],
                                                 scale=1.0, accum_out=ssum[:msz])
                            rinv = smxp.tile([128, 1], F32, tag="rinv",
                                             name="rinv")
                            nc.vector.reciprocal(out=rinv[:msz], in_=ssum[:msz])
                            nc.vector.tensor_scalar_mul(out=o[:msz, :],
                                                        in0=o[:msz, :],
                                                        scalar1=rinv[:msz])
                        else:
                            nc.vector.tensor_copy(out=o[:msz, :], in_=p[:msz, :])
                        outs.append(o)
                    return outs

                def pair21(lhs_kt, rhs_kt, m, n, tag, engine="v"):
                    """out[m,n] = sum_kt lhs_kt^T @ rhs_kt (2 k-tiles)"""
                    p = psS.tile([m, n], F32, tag="mm", name="p")
                    for ki, (k0, ksz) in enumerate(_M2):
                        nc.tensor.matmul(p[:], lhsT=lhs_kt[ki][:ksz, :m],
                                         rhs=rhs_kt[ki][:ksz, :n],
                                         start=(ki == 0), stop=(ki == 1))
                    o = l2p.tile([m, n], F32, tag=tag, name="o")
                    if engine == "v":
                        nc.vector.tensor_copy(out=o[:], in_=p[:])
                    else:
                        nc.scalar.activation(out=o[:], in_=p[:], func=AF.Copy,
                                             scale=1.0)
                    T[tag] = o
                    return o

                def s1():
                    T["z1s"] = wmm205(w2ap("Wp1"), K2, "z1s")
                def s2():
                    T["z1e"] = wmm205(w2ap("We1"), HID, "z1e")
                def s3():
                    hhT(T["z1s"], "Up1", K2, "hhts")
                def s4():
                    hhT(T["z1e"], "Ue1", HID, "hhte")
                def s5():
                    T["z2s"] = z2s_m(T["hhts"], "Wp2", K2, "z2s")
                def s6():
                    T["z2e"] = z2s_m(T["hhte"], "We2", HID, "z2e")
                def s7():
                    T["sm2"] = stage_o(T["z2s"], T["hhts"], "Up2", K2, "sm2",
                                       softmax=True)
                def s8():
                    T["x1e"] = stage_o(T["z2e"], T["hhte"], "Ue2", HID, "x1e",
                                       softmax=False)
                def s9():
                    outs = []
                    for mi, (m0, msz) in enumerate(_M2):
                        p = psS.tile([128, K2], F32, tag="mm", name="p")
                        for ki, (k0, ksz) in enumerate(_M2):
                            nc.tensor.matmul(p[:msz, :],
                                             lhsT=at[ki][:ksz, m0:m0 + msz],
                                             rhs=T["sm2"][ki][:ksz, :],
                                             start=(ki == 0), stop=(ki == 1))
                        o = l2p.tile([128, K2], F32, tag=f"y{mi}", name="o")
                        nc.vector.tensor_copy(out=o[:msz, :], in_=p[:msz, :])
                        outs.append(o)
                    T["y"] = outs
                def s10():
                    pair21(T["x1e"], T["sm2"], HID, K2, "x2t", engine="s")
                def s11():
                    # a2t copy via ACT with accum_out: row sums of a2t are the
                    # column sums of a2, needed for the fused mean readout
                    p = psS.tile([K2, K2], F32, tag="mm", name="p")
                    for ki, (k0, ksz) in enumerate(_M2):
                        nc.tensor.matmul(p[:], lhsT=T["y"][ki][:ksz, :K2],
                                         rhs=T["sm2"][ki][:ksz, :K2],
                                         start=(ki == 0), stop=(ki == 1))
                    o = l2p.tile([K2, K2], F32, tag="a2t", name="o")
                    ra = l2p.tile([K2, 1], F32, tag="ra2t", name="ra")
                    nc.scalar.activation(out=o[:], in_=p[:], func=AF.Copy,
                                         scale=1.0, accum_out=ra[:])
                    T["a2t"] = o
                    T["ra2t"] = ra
                def s12():
                    p = psS.tile([K2, HID], F32, tag="mm", name="p")
                    nc.tensor.matmul(p[:], lhsT=T["x2t"][:HID, :K2],
                                     rhs=w2ap("Wc1"), start=True, stop=True)
                    o = l2p.tile([K2, HID], F32, tag="zf", name="o")
                    nc.vector.tensor_copy(out=o[:], in_=p[:])
                    T["zf"] = o
                def s13():
                    # h2t relu with accum_out: row sums of h2t = 1^T h2,
                    # needed for the fused mean readout
                    p = psS.tile([HID, K2], F32, tag="mm", name="p")
                    nc.tensor.matmul(p[:], lhsT=T["zf"][:K2, :HID],
                                     rhs=T["a2t"][:K2, :K2],
                                     start=True, stop=False)
                    nc.tensor.matmul(p[:], lhsT=w2ap("Uc1"),
                                     rhs=T["x2t"][:HID, :K2],
                                     start=False, stop=True)
                    o = l2p.tile([HID, K2], F32, tag="h2t", name="o")
                    rh = l2p.tile([HID, 1], F32, tag="rh2t", name="rh")
                    nc.scalar.activation(out=o[:], in_=p[:], func=AF.Relu,
                                         accum_out=rh[:])
                    T["h2t"] = o
                    T["rh2t"] = rh
                def s14():
                    p = psS.tile([K2, OUT], F32, tag="mm", name="p")
                    nc.tensor.matmul(p[:], lhsT=T["h2t"][:HID, :K2],
                                     rhs=w2ap("Wc2"), start=True, stop=True)
                    o = l2p.tile([K2, OUT], F32, tag="z2f", name="o")
                    nc.vector.tensor_copy(out=o[:], in_=p[:])
                    T["z2f"] = o
                def s15():
                    # fused readout: mean_m(a2 @ z2f + h2 @ Uc2)[m, :] / 21
                    # = (ra2^T @ z2f + rh2^T @ Uc2) / 21
                    p = psS.tile([1, OUT], F32, tag="mm", name="p")
                    nc.tensor.matmul(p[:], lhsT=T["ra2t"][:K2, :1],
                                     rhs=T["z2f"][:K2, :OUT],
                                     start=True, stop=False)
                    nc.tensor.matmul(p[:], lhsT=T["rh2t"][:HID, :1],
                                     rhs=w2ap("Uc2"), start=False, stop=True)
                    nc.scalar.activation(out=out_sbs[b][:], in_=p[:],
                                         func=AF.Copy, scale=1.0 / K2)

                stages = [s1, s2, s3, s4, s5, s6, s7, s8, s9, s10, s11, s12,
                          s13, s14, s15]
                if debug and b == 0:
                    def dump():
                        for mi, (m0, msz) in enumerate(_M2):
                            nc.sync.dma_start(out=dbg["a1t"][m0:m0 + msz, :],
                                              in_=at[mi][:msz, :])
                            nc.sync.dma_start(out=dbg["sm2"][m0:m0 + msz, :],
                                              in_=T["sm2"][mi][:msz, :])
                            nc.sync.dma_start(out=dbg["x1e"][m0:m0 + msz, :],
                                              in_=T["x1e"][mi][:msz, :])
                        nc.sync.dma_start(out=dbg["x1t"][:], in_=x1t[b][:])
                        nc.sync.dma_start(out=dbg["x2t"][:], in_=T["x2t"][:])
                        nc.sync.dma_start(out=dbg["a2t"][:], in_=T["a2t"][:])
                    stages = stages[:11] + [dump] + stages[11:]
                return stages

            # ================= emission =================
            proj(0)
            proj(1)
            emit_adj_dmas(0)
            emit_adj_dmas(1)

            if debug:
                for i in range(16):
                    scp = l2p.tile([128, K1], F32, tag="dbgcp", name="scp")
                    nc.vector.tensor_copy(out=scp[:], in_=s_bf[0][i][:])
                    nc.sync.dma_start(out=dbg["s"][i * 128:(i + 1) * 128, :],
                                      in_=scp[:])

            # batch 0 level-1
            taccs0 = []
            for v in range(16):
                passA_v(0, v, taccs0)
            for u in range(NA, 16):
                passB_u(0, u)
            a1t_m(0, 0)
            a1t_m(0, 1)
            x1t_u(0)

            if debug:
                for u in range(16):
                    tcp = l2p.tile([128, K1], F32, tag="dbgcp", name="tcp")
                    nc.vector.tensor_copy(out=tcp[:], in_=t_bf[0][u][:])
                    nc.sync.dma_start(out=dbg["t"][u * 128:(u + 1) * 128, :],
                                      in_=tcp[:])

            # batch 1 level-1 with batch 0 level-2 stages 1..8 interleaved;
            # batch 0's remaining stages pair with batch 1's level-2 tail so
            # the tail's dependency stalls are filled with independent work
            stages0 = lvl2_stages(0)
            si = 0
            nunits = 29
            uidx = 0

            def sprinkle():
                nonlocal si, uidx
                uidx += 1
                while si < min(8, (uidx * 8 + nunits - 1) // nunits):
                    stages0[si]()
                    si += 1

            taccs1 = []
            for v in range(16):
                passA_v(1, v, taccs1)
                sprinkle()
            for u in range(NA, 16):
                passB_u(1, u)
                sprinkle()
            a1t_m(1, 0)
            sprinkle()
            a1t_m(1, 1)
            sprinkle()
            x1t_u(1)

            for st1 in lvl2_stages(1):
                if si < len(stages0):
                    stages0[si]()
                    si += 1
                st1()
            while si < len(stages0):
                stages0[si]()
                si += 1

            for b in range(BPC):
                nc.sync.dma_start(out=out[b:b + 1, :], in_=out_sbs[b][:])

    _legalize_multiwait(nc)
    return nc


# ---------------------------------------------------------------------------
# Host side
# ---------------------------------------------------------------------------
def _prep_inputs(inputs):
    inp = {k: np.asarray(v) for k, v in inputs.items()}
    sl1 = inp["slice_g1"].astype(np.int64)
    sl2 = inp["slice_g2"].astype(np.int64)
    b1 = inp["batch_g1"].astype(np.int64)
    b2 = inp["batch_g2"].astype(np.int64)
    n1 = np.diff(sl1)
    assert (n1 == N1P).all() and (np.diff(sl2) == N2P).all(), \
        "kernel hardcodes n1=1100/n2=900 per batch"
    pos1 = np.arange(inp["x_g1"].shape[0], dtype=np.int64) - sl1[b1]
    pos2 = (np.arange(inp["x_g2"].shape[0], dtype=np.int64) - sl2[b2]
            + n1[b2])

    # packed dense transposed features, bf16: cols 0:1152 hold g1 features at
    # node position, cols 1152:2176 hold g2 features at position-1024
    xtp = np.zeros((B, IN_DIM, XTC), np.float32)
    xg1t = inp["x_g1"].T
    xg2t = inp["x_g2"].T
    for b in range(B):
        r1 = slice(sl1[b], sl1[b + 1])
        xtp[b][:, pos1[r1]] = xg1t[:, r1]
        r2 = slice(sl2[b], sl2[b + 1])
        xtp[b][:, WTRIM + pos2[r2] - 1024] = xg2t[:, r2]
    xtp = xtp.astype(BFNP)

    # packed transposed dense adjacency [B, 128, TOTC]: partition = dst&127,
    # column = OFFV[dst>>7] + src; v-blocks 0..7 (dst<1024 => g1 dst =>
    # src < 1152) are width-trimmed. Per-partition rows are contiguous.
    e1, e2, eh = inp["edge_g1"], inp["edge_g2"], inp["edge_h"]
    eb = np.concatenate([b1[e1[0]], b2[e2[0]], b1[eh[0]]]).astype(np.int64)
    src = np.concatenate([pos1[e1[0]], pos2[e2[0]], pos1[eh[0]]])
    dst = np.concatenate([pos1[e1[1]], pos2[e2[1]], pos2[eh[1]]])
    vsl = dst >> 7
    offv = np.where(vsl < 8, vsl * WTRIM, 8 * WTRIM + (vsl - 8) * MN)
    if ADJ_FP8:
        adj_u = np.zeros((B, 128 * TOTC), np.uint8)
    else:
        adj_u = np.zeros((B, 128 * TOTC), np.uint16)
    adj_u[eb, (dst & 127) * TOTC + offv + src] = ADJ_ONE
    adj_np = adj_u.view(ADJ_NP).reshape(B, 128, TOTC)

    # fused projection weights [128, 538] bf16
    wallh = np.concatenate(
        [inp["W_pool_g1"], inp["W_emb_g1"], inp["W_pool_g2"], inp["W_emb_g2"]],
        axis=1).astype(np.float32).astype(BFNP)
    # packed level-2 weights [64, 472] f32
    w2h = np.zeros((HID, W2W), np.float32)
    for name, (rows, c0, c1) in _W2COLS.items():
        w2h[:rows, c0:c1] = inp[name]

    in_maps = []
    for c in range(NCORES):
        bs = slice(c * BPC, (c + 1) * BPC)
        in_maps.append(dict(
            xt=np.ascontiguousarray(xtp[bs]),
            adj=np.ascontiguousarray(adj_np[bs]),
            wall=wallh, w2=w2h,
        ))
    return in_maps


_NC_CACHE = {}


def run(inputs, debug=False, trace=False, tmpdir=None):
    key = bool(debug)
    if key not in _NC_CACHE:
        _NC_CACHE[key] = build_nc(debug=debug)
    nc = _NC_CACHE[key]
    in_maps = _prep_inputs(inputs)
    res = run_bass_kernel_spmd(nc, in_maps, list(range(NCORES)),
                               trace=trace, tmpdir=tmpdir)
    y = np.zeros((B, OUT), np.float32)
    for c in range(NCORES):
        o = res.results[c]["out"]       # [BPC, OUT]
        for b in range(BPC):
            y[c * BPC + b] = o[b]
    return y, res


def kernel(**inputs):
    y, _ = run(inputs)
    return y


# revision 28
# speedup vs baseline: 1.1997x; 1.1997x over previous
"""DiffPool GNN forward on 8 Trainium2 NeuronCores.

Data-parallel over the batch dim (B=16 -> 2 batches per core). Host packs
per-batch dense transposed features (bf16) and a column-trimmed transposed
dense adjacency (bf16); each core runs the DiffPool batched GEMMs locally.

Structure per core (emission order = per-engine execution order):
  warmup MMs (HAM un-throttle) -> proj b0, proj b1 (bf16, fused pool|emb
  weights) -> b0 t-GEMM pass A (6 PSUM banks, v-outer, consumes adjacency
  slabs as they stream) + pass B (u-outer) -> a1t/x1t b0 -> b1 passes with
  the level-2 stages of b0 interleaved between v/u-groups -> level-2 b1.

Level-2 is restructured transpose-free: every product is emitted in the
orientation whose result is directly the lhsT of its consumer, so no
PE-transpose / copy pairs are needed. Level-2 stays f32 (softmax of s2 is
sensitive); level-1 operands are bf16.
"""

import numpy as np
import ml_dtypes

import concourse.bass as bass
import concourse.mybir as mybir
from concourse import tile
from concourse.bass_utils import run_bass_kernel_spmd

# ---------------------------------------------------------------------------
# Problem constants (hardcoded per spec; setup_inputs has n1=1100, n2=900)
# ---------------------------------------------------------------------------
B = 16
NCORES = 8
BPC = B // NCORES          # batches per core
MN = 2048                  # MAX_NODES
IN_DIM = 128
HID = 64
OUT = 2
K1 = 205
K2 = 21
N1P = 1100                 # g1 nodes per batch (constant in generator)
N2P = 900
WTRIM = 1152               # trimmed slab width: cols (src) kept for v-slabs 0..7
NPROJ = K1 + HID           # fused pool|emb projection width

F32 = mybir.dt.float32
BF16 = mybir.dt.bfloat16
F8 = mybir.dt.float8e4
AF = mybir.ActivationFunctionType
BFNP = ml_dtypes.bfloat16
F8NP = ml_dtypes.float8_e4m3

# adjacency dtype: fp8 halves HBM traffic; entries are exactly 0/1 so the
# only risk is tensor-engine support for fp8 weights x bf16 moving operand
ADJ_FP8 = True
ADJ_DT = F8 if ADJ_FP8 else BF16
ADJ_ONE = 0x38 if ADJ_FP8 else 0x3F80
ADJ_NP = F8NP if ADJ_FP8 else BFNP

# packed adjacency layout: per batch [128, TOTC]; partition = dst&127,
# column block v = dst>>7 at OFFV[v], col within block = src (trimmed to
# 1152 for v<8 where dst is a g1 node so src < 1152)
WIDV = [WTRIM] * 8 + [MN] * 8
OFFV = [0] * 16
for _v in range(1, 16):
    OFFV[_v] = OFFV[_v - 1] + WIDV[_v - 1]
TOTC = OFFV[15] + WIDV[15]             # 25600
XTC = WTRIM + 1024                     # packed features width 2176

_M2 = ((0, 128), (128, K1 - 128))      # row tiling of 205-row matrices

# level-2 weight pack layout: [64, 472] f32
_W2COLS = {
    "Wp1": (64, 0, 21), "Up1": (64, 21, 42), "Wp2": (21, 42, 63),
    "Up2": (21, 63, 84), "We1": (64, 84, 148), "Ue1": (64, 148, 212),
    "We2": (64, 212, 276), "Ue2": (64, 276, 340), "Wc1": (64, 340, 404),
    "Uc1": (64, 404, 468), "Wc2": (64, 468, 470), "Uc2": (64, 470, 472),
}
W2W = 472


# ---------------------------------------------------------------------------
# Walrus workaround: this toolchain's walrus encodes at most ONE sync wait
# per instruction; split multi-wait instructions via single-wait NOPs.
# ---------------------------------------------------------------------------
_mw_ctr = [0]


def _legalize_multiwait(nc):
    for func in nc.m.functions:
        for bb in func.blocks:
            insts = bb.instructions
            new = []
            changed = False
            for ins in insts:
                si = getattr(ins, "sync_info", None)
                waits = list(si.on_wait) if (si and si.on_wait) else []
                if len(waits) > 1:
                    changed = True
                    for w in waits[:-1]:
                        _mw_ctr[0] += 1
                        nop = mybir.InstNoOp(
                            name=f"mwfix-{_mw_ctr[0]}",
                            engine=ins.engine,
                            ins=[],
                            outs=[],
                            sync_info=mybir.SyncInfo(on_wait=[w], on_update=[]),
                            bass_nofuse=True,
                        )
                        nc.register_instruction(nop, overwrite=True)
                        new.append(nop)
                    si.on_wait = [waits[-1]]
                new.append(ins)
            if changed:
                bb.instructions[:] = new


# ---------------------------------------------------------------------------
# Device program
# ---------------------------------------------------------------------------
def build_nc(debug=False):
    nc = bass.Bass()

    xt = nc.dram_tensor("xt", [BPC, IN_DIM, XTC], BF16, kind="ExternalInput")
    adj = nc.dram_tensor("adj", [BPC, 128, TOTC], ADJ_DT,
                         kind="ExternalInput")
    wall = nc.dram_tensor("wall", [IN_DIM, 2 * NPROJ], BF16,
                          kind="ExternalInput")
    w2 = nc.dram_tensor("w2", [HID, W2W], F32, kind="ExternalInput")
    out = nc.dram_tensor("out", [BPC, OUT], F32, kind="ExternalOutput")
    if debug:
        dbg = {}
        for nm, shp in [("s", [MN, K1]), ("t", [MN, K1]), ("a1t", [K1, K1]),
                        ("x1t", [HID, K1]), ("sm2", [K1, K2]),
                        ("x1e", [K1, HID]), ("x2t", [HID, K2]),
                        ("a2t", [K2, K2])]:
            dbg[nm] = nc.dram_tensor(f"dbg_{nm}", shp, F32,
                                     kind="ExternalOutput")

    with tile.TileContext(nc) as tc:
        with (
            tc.tile_pool(name="const", bufs=1) as cpool,
            tc.tile_pool(name="xtp", bufs=2) as xtp,
            tc.tile_pool(name="slab", bufs=2) as slabp,
            tc.tile_pool(name="act", bufs=1) as actp,
            tc.tile_pool(name="tt", bufs=2) as tp,
            tc.tile_pool(name="l2", bufs=2) as l2p,
            tc.tile_pool(name="smx", bufs=3) as smxp,
            tc.tile_pool(name="psA", bufs=6, space="PSUM") as psA,
            tc.tile_pool(name="psS", bufs=2, space="PSUM") as psS,
        ):
            # ---- constants / weights ----
            out_sbs = [cpool.tile([1, OUT], F32, tag=f"out_sb{b}",
                                  name="osb") for b in range(BPC)]

            wall_sb = cpool.tile([IN_DIM, 2 * NPROJ], BF16, tag="wall")
            nc.sync.dma_start(out=wall_sb[:], in_=wall[:])
            w2_sb = cpool.tile([HID, W2W], F32, tag="w2")
            nc.sync.dma_start(out=w2_sb[:], in_=w2[:])

            # warmup: PE busy from t~8us so the HAM clock gate opens before
            # the projections start (it needs ~3.4us of sustained activity)
            warm = cpool.tile([128, 256], BF16, tag="warm")
            nc.gpsimd.memset(warm[:], 0.0)
            for _ in range(6):
                pw = psS.tile([128, 256], F32, tag="mm", name="pw")
                nc.tensor.matmul(pw[:], lhsT=warm[:, :128], rhs=warm[:],
                                 start=True, stop=True)

            def w2ap(name):
                rows, c0, c1 = _W2COLS[name]
                return w2_sb[:rows, c0:c1]

            # ---- per-batch state ----
            s_bf = [[None] * 16, [None] * 16]
            h_bf = [[None] * 16, [None] * 16]
            adj_sb = [None, None]
            t_bf = [[None] * 16, [None] * 16]
            a1t = [[None, None], [None, None]]
            x1t = [None, None]

            # ---- projections: s = softmax(x@Wpool), h = relu(x@Wemb) ----
            # both feature DMAs issued upfront, before the adjacency stream
            xsbs = []
            for b in range(BPC):
                xsb = xtp.tile([IN_DIM, XTC], BF16, tag="xt", name="xsb")
                nc.sync.dma_start(out=xsb[:], in_=xt[b])
                xsbs.append(xsb)

            def proj(b):
                xsb = xsbs[b]
                for i in range(16):
                    pp = psS.tile([128, NPROJ], F32, tag="mm", name="pp")
                    if i < 8:
                        nc.tensor.matmul(pp[:], lhsT=xsb[:, i * 128:(i + 1) * 128],
                                         rhs=wall_sb[:, :NPROJ],
                                         start=True, stop=True)
                    elif i == 8:
                        nc.tensor.matmul(pp[:], lhsT=xsb[:, 1024:WTRIM],
                                         rhs=wall_sb[:, :NPROJ],
                                         start=True, stop=False)
                        nc.tensor.matmul(pp[:], lhsT=xsb[:, WTRIM:WTRIM + 128],
                                         rhs=wall_sb[:, NPROJ:],
                                         start=False, stop=True)
                    else:
                        c0 = 128 * (i + 1)
                        nc.tensor.matmul(pp[:], lhsT=xsb[:, c0:c0 + 128],
                                         rhs=wall_sb[:, NPROJ:],
                                         start=True, stop=True)
                    # softmax without max-subtract: |scores| < 0.2 by scale
                    st = actp.tile([128, K1], BF16, tag=f"s{b}_{i}", name="st")
                    ssum = smxp.tile([128, 1], F32, tag="ssum", name="ssum")
                    nc.scalar.activation(out=st[:], in_=pp[:, :K1],
                                         func=AF.Exp, scale=1.0,
                                         accum_out=ssum[:])
                    rinv = smxp.tile([128, 1], F32, tag="rinv", name="rinv")
                    nc.vector.reciprocal(out=rinv[:], in_=ssum[:])
                    nc.vector.tensor_scalar_mul(out=st[:], in0=st[:],
                                                scalar1=rinv[:])
                    ht = actp.tile([128, HID], BF16, tag=f"h{b}_{i}", name="ht")
                    nc.scalar.activation(out=ht[:], in_=pp[:, K1:],
                                         func=AF.Relu)
                    s_bf[b][i] = st
                    h_bf[b][i] = ht

            # adjacency DMA: 4 column-group transfers per batch, each into its
            # own tile (distinct dep per group) so pass A starts on group 0;
            # rows are long contiguous runs (fast DMA)
            ADJ_GROUPS = [(OFFV[0], OFFV[4]), (OFFV[4], OFFV[8]),
                          (OFFV[8], OFFV[12]), (OFFV[12], TOTC)]

            def emit_adj_dmas(b):
                tiles = []
                for gi, (c0, c1) in enumerate(ADJ_GROUPS):
                    g = slabp.tile([128, c1 - c0], ADJ_DT, tag=f"adjg{gi}",
                                   name="g")
                    nc.sync.dma_start(out=g[:], in_=adj[b][:, c0:c1])
                    tiles.append(g)
                adj_sb[b] = tiles

            def adj_ap(b, v, u):
                gi = v // 4
                c0 = OFFV[v] - ADJ_GROUPS[gi][0] + u * 128
                return adj_sb[b][gi][:, c0:c0 + 128]

            # ---- t = adj @ s (u-blocked into PSUM banks; skip the zero
            #      block: u-slabs 9..15 x v-slabs 0..7 of adj are zero) ----
            NA = 6   # pass-A u-group width = psA bufs

            def cast_t(b, u, tacc):
                tt = tp.tile([128, K1], BF16, tag=f"t{u}", name="tt")
                if u % 2 == 0:
                    nc.vector.tensor_copy(out=tt[:], in_=tacc[:])
                else:
                    nc.scalar.activation(out=tt[:], in_=tacc[:], func=AF.Copy,
                                         scale=1.0)
                t_bf[b][u] = tt

            def passA_v(b, v, taccs):
                if v == 0:
                    for u in range(NA):
                        taccs.append(psA.tile([128, K1], F32, tag="tacc",
                                              name="tacc"))
                for u in range(NA):
                    nc.tensor.matmul(taccs[u][:],
                                     lhsT=adj_ap(b, v, u),
                                     rhs=s_bf[b][v][:],
                                     start=(v == 0), stop=(v == 15))
                if v == 15:
                    for u in range(NA):
                        cast_t(b, u, taccs[u])

            def passB_u(b, u):
                vs = list(range(16)) if u <= 8 else list(range(8, 16))
                tacc = psA.tile([128, K1], F32, tag="tacc", name="tacc")
                for v in vs:
                    nc.tensor.matmul(tacc[:],
                                     lhsT=adj_ap(b, v, u),
                                     rhs=s_bf[b][v][:],
                                     start=(v == vs[0]), stop=(v == vs[-1]))
                cast_t(b, u, tacc)

            # ---- a1t = t^T s  [205,205] (row-tiled), x1t = h^T s [64,205] ----
            def a1t_m(b, mi):
                m0, msz = _M2[mi]
                pa = psS.tile([128, K1], F32, tag="mm", name="pa")
                for v in range(16):
                    nc.tensor.matmul(pa[:msz, :],
                                     lhsT=t_bf[b][v][:, m0:m0 + msz],
                                     rhs=s_bf[b][v][:],
                                     start=(v == 0), stop=(v == 15))
                asb = l2p.tile([128, K1], F32, tag=f"a1t{mi}", name="asb")
                nc.vector.tensor_copy(out=asb[:msz, :], in_=pa[:msz, :])
                a1t[b][mi] = asb

            def x1t_u(b):
                px = psS.tile([HID, K1], F32, tag="mm", name="px")
                for v in range(16):
                    nc.tensor.matmul(px[:], lhsT=h_bf[b][v][:],
                                     rhs=s_bf[b][v][:],
                                     start=(v == 0), stop=(v == 15))
                xsb = l2p.tile([HID, K1], F32, tag="x1t", name="xsb")
                nc.vector.tensor_copy(out=xsb[:], in_=px[:])
                x1t[b] = xsb

            # ---- level-2: transpose-free stage list ----
            def lvl2_stages(b):
                at, xt_ = a1t[b], x1t[b]
                T = {}

                def wmm205(rhs_ap, n, tag, relu=False):
                    """out[205,n] = x1 @ W as 2 row-tiles: lhsT=x1t col-slice"""
                    outs = []
                    for mi, (m0, msz) in enumerate(_M2):
                        p = psS.tile([128, n], F32, tag="mm", name="p")
                        nc.tensor.matmul(p[:msz, :], lhsT=xt_[:, m0:m0 + msz],
                                         rhs=rhs_ap, start=True, stop=True)
                        o = l2p.tile([128, n], F32, tag=f"{tag}{mi}", name="o")
                        nc.vector.tensor_copy(out=o[:msz, :], in_=p[:msz, :])
                        outs.append(o)
                    return outs

                def hhT(z1, U1, n, tag):
                    """hhT[n,205] = relu((a1 @ z1 + x1 @ U1)^T)"""
                    p = psS.tile([n, K1], F32, tag="mm", name="p")
                    for ki, (k0, ksz) in enumerate(_M2):
                        nc.tensor.matmul(p[:], lhsT=z1[ki][:ksz, :n],
                                         rhs=at[ki][:ksz, :],
                                         start=(ki == 0), stop=False)
                    nc.tensor.matmul(p[:], lhsT=w2ap(U1)[:, :n], rhs=xt_[:],
                                     start=False, stop=True)
                    o = l2p.tile([n, K1], F32, tag=tag, name="o")
                    nc.scalar.activation(out=o[:], in_=p[:], func=AF.Relu)
                    T[tag] = o
                    return o

                def z2s_m(hh, W2n, n, tag):
                    """z2[205,n] = hh @ W2 as row-tiles: lhsT=hhT col-slice"""
                    outs = []
                    nh = hh.shape[0]
                    for mi, (m0, msz) in enumerate(_M2):
                        p = psS.tile([128, n], F32, tag="mm", name="p")
                        nc.tensor.matmul(p[:msz, :], lhsT=hh[:nh, m0:m0 + msz],
                                         rhs=w2ap(W2n)[:nh, :n],
                                         start=True, stop=True)
                        o = l2p.tile([128, n], F32, tag=f"{tag}{mi}", name="o")
                        nc.scalar.activation(out=o[:msz, :], in_=p[:msz, :],
                                             func=AF.Copy, scale=1.0)
                        outs.append(o)
                    return outs

                def stage_o(z2, hh, U2, n, tag, softmax):
                    """o[205,n] = a1 @ z2 + hh @ U2, per row-tile; optionally
                    softmax along free dim into tag tiles."""
                    outs = []
                    nh = hh.shape[0]
                    for mi, (m0, msz) in enumerate(_M2):
                        p = psS.tile([128, n], F32, tag="mm", name="p")
                        for ki, (k0, ksz) in enumerate(_M2):
                            nc.tensor.matmul(p[:msz, :],
                                             lhsT=at[ki][:ksz, m0:m0 + msz],
                                             rhs=z2[ki][:ksz, :],
                                             start=(ki == 0), stop=False)
                        nc.tensor.matmul(p[:msz, :],
                                         lhsT=hh[:nh, m0:m0 + msz],
                                         rhs=w2ap(U2)[:nh, :n],
                                         start=False, stop=True)
                        o = l2p.tile([128, n], F32, tag=f"{tag}{mi}", name="o")
                        if softmax:
                            nmax = smxp.tile([128, 1], F32, tag="nmax",
                                             name="nmax")
                            nc.vector.reduce_max(out=nmax[:msz], in_=p[:msz, :],
                                                 axis=mybir.AxisListType.X,
                                                 negate=True)
                            ssum = smxp.tile([128, 1], F32, tag="ssum",
                                             name="ssum")
                            nc.scalar.activation(out=o[:msz, :], in_=p[:msz, :],
                                                 func=AF.Exp, bias=nmax[:msz],
                                                 scale=1.0, accum_out=ssum[:msz])
                            rinv = smxp.tile([128, 1], F32, tag="rinv",
                                             name="rinv")
                            nc.vector.reciprocal(out=rinv[:msz], in_=ssum[:msz])
                            nc.vector.tensor_scalar_mul(out=o[:msz, :],
                                                        in0=o[:msz, :],
                                                        scalar1=rinv[:msz])
                        else:
                            nc.vector.tensor_copy(out=o[:msz, :], in_=p[:msz, :])
                        outs.append(o)
                    return outs

                def pair21(lhs_kt, rhs_kt, m, n, tag, engine="v"):
                    """out[m,n] = sum_kt lhs_kt^T @ rhs_kt (2 k-tiles)"""
                    p = psS.tile([m, n], F32, tag="mm", name="p")
                    for ki, (k0, ksz) in enumerate(_M2):
                        nc.tensor.matmul(p[:], lhsT=lhs_kt[ki][:ksz, :m],
                                         rhs=rhs_kt[ki][:ksz, :n],
                                         start=(ki == 0), stop=(ki == 1))
                    o = l2p.tile([m, n], F32, tag=tag, name="o")
                    if engine == "v":
                        nc.vector.tensor_copy(out=o[:], in_=p[:])
                    else:
                        nc.scalar.activation(out=o[:], in_=p[:], func=AF.Copy,
                                             scale=1.0)
                    T[tag] = o
                    return o

                def s1():
                    T["z1s"] = wmm205(w2ap("Wp1"), K2, "z1s")
                def s2():
                    T["z1e"] = wmm205(w2ap("We1"), HID, "z1e")
                def s3():
                    hhT(T["z1s"], "Up1", K2, "hhts")
                def s4():
                    hhT(T["z1e"], "Ue1", HID, "hhte")
                def s5():
                    T["z2s"] = z2s_m(T["hhts"], "Wp2", K2, "z2s")
                def s6():
                    T["z2e"] = z2s_m(T["hhte"], "We2", HID, "z2e")
                def s7():
                    T["sm2"] = stage_o(T["z2s"], T["hhts"], "Up2", K2, "sm2",
                                       softmax=True)
                def s8():
                    T["x1e"] = stage_o(T["z2e"], T["hhte"], "Ue2", HID, "x1e",
                                       softmax=False)
                def s9():
                    outs = []
                    for mi, (m0, msz) in enumerate(_M2):
                        p = psS.tile([128, K2], F32, tag="mm", name="p")
                        for ki, (k0, ksz) in enumerate(_M2):
                            nc.tensor.matmul(p[:msz, :],
                                             lhsT=at[ki][:ksz, m0:m0 + msz],
                                             rhs=T["sm2"][ki][:ksz, :],
                                             start=(ki == 0), stop=(ki == 1))
                        o = l2p.tile([128, K2], F32, tag=f"y{mi}", name="o")
                        nc.vector.tensor_copy(out=o[:msz, :], in_=p[:msz, :])
                        outs.append(o)
                    T["y"] = outs
                def s10():
                    pair21(T["x1e"], T["sm2"], HID, K2, "x2t", engine="s")
                def s11():
                    # a2t copy via ACT with accum_out: row sums of a2t are the
                    # column sums of a2, needed for the fused mean readout
                    p = psS.tile([K2, K2], F32, tag="mm", name="p")
                    for ki, (k0, ksz) in enumerate(_M2):
                        nc.tensor.matmul(p[:], lhsT=T["y"][ki][:ksz, :K2],
                                         rhs=T["sm2"][ki][:ksz, :K2],
                                         start=(ki == 0), stop=(ki == 1))
                    o = l2p.tile([K2, K2], F32, tag="a2t", name="o")
                    ra = l2p.tile([K2, 1], F32, tag="ra2t", name="ra")
                    nc.scalar.activation(out=o[:], in_=p[:], func=AF.Copy,
                                         scale=1.0, accum_out=ra[:])
                    T["a2t"] = o
                    T["ra2t"] = ra
                def s12():
                    p = psS.tile([K2, HID], F32, tag="mm", name="p")
                    nc.tensor.matmul(p[:], lhsT=T["x2t"][:HID, :K2],
                                     rhs=w2ap("Wc1"), start=True, stop=True)
                    o = l2p.tile([K2, HID], F32, tag="zf", name="o")
                    nc.vector.tensor_copy(out=o[:], in_=p[:])
                    T["zf"] = o
                def s13():
                    # h2t relu with accum_out: row sums of h2t = 1^T h2,
                    # needed for the fused mean readout
                    p = psS.tile([HID, K2], F32, tag="mm", name="p")
                    nc.tensor.matmul(p[:], lhsT=T["zf"][:K2, :HID],
                                     rhs=T["a2t"][:K2, :K2],
                                     start=True, stop=False)
                    nc.tensor.matmul(p[:], lhsT=w2ap("Uc1"),
                                     rhs=T["x2t"][:HID, :K2],
                                     start=False, stop=True)
                    o = l2p.tile([HID, K2], F32, tag="h2t", name="o")
                    rh = l2p.tile([HID, 1], F32, tag="rh2t", name="rh")
                    nc.scalar.activation(out=o[:], in_=p[:], func=AF.Relu,
                                         accum_out=rh[:])
                    T["h2t"] = o
                    T["rh2t"] = rh
                def s14():
                    p = psS.tile([K2, OUT], F32, tag="mm", name="p")
                    nc.tensor.matmul(p[:], lhsT=T["h2t"][:HID, :K2],
                                     rhs=w2ap("Wc2"), start=True, stop=True)
                    o = l2p.tile([K2, OUT], F32, tag="z2f", name="o")
                    nc.vector.tensor_copy(out=o[:], in_=p[:])
                    T["z2f"] = o
                def s15():
                    # fused readout: mean_m(a2 @ z2f + h2 @ Uc2)[m, :] / 21
                    # = (ra2^T @ z2f + rh2^T @ Uc2) / 21
                    p = psS.tile([1, OUT], F32, tag="mm", name="p")
                    nc.tensor.matmul(p[:], lhsT=T["ra2t"][:K2, :1],
                                     rhs=T["z2f"][:K2, :OUT],
                                     start=True, stop=False)
                    nc.tensor.matmul(p[:], lhsT=T["rh2t"][:HID, :1],
                                     rhs=w2ap("Uc2"), start=False, stop=True)
                    nc.scalar.activation(out=out_sbs[b][:], in_=p[:],
                                         func=AF.Copy, scale=1.0 / K2)

                stages = [s1, s2, s3, s4, s5, s6, s7, s8, s9, s10, s11, s12,
                          s13, s14, s15]
                if debug and b == 0:
                    def dump():
                        for mi, (m0, msz) in enumerate(_M2):
                            nc.sync.dma_start(out=dbg["a1t"][m0:m0 + msz, :],
                                              in_=at[mi][:msz, :])
                            nc.sync.dma_start(out=dbg["sm2"][m0:m0 + msz, :],
                                              in_=T["sm2"][mi][:msz, :])
                            nc.sync.dma_start(out=dbg["x1e"][m0:m0 + msz, :],
                                              in_=T["x1e"][mi][:msz, :])
                        nc.sync.dma_start(out=dbg["x1t"][:], in_=x1t[b][:])
                        nc.sync.dma_start(out=dbg["x2t"][:], in_=T["x2t"][:])
                        nc.sync.dma_start(out=dbg["a2t"][:], in_=T["a2t"][:])
                    stages = stages[:11] + [dump] + stages[11:]
                return stages

            # ================= emission =================
            proj(0)
            proj(1)
            emit_adj_dmas(0)
            emit_adj_dmas(1)

            if debug:
                for i in range(16):
                    scp = l2p.tile([128, K1], F32, tag="dbgcp", name="scp")
                    nc.vector.tensor_copy(out=scp[:], in_=s_bf[0][i][:])
                    nc.sync.dma_start(out=dbg["s"][i * 128:(i + 1) * 128, :],
                                      in_=scp[:])

            # batch 0 level-1
            taccs0 = []
            for v in range(16):
                passA_v(0, v, taccs0)
            for u in range(NA, 16):
                passB_u(0, u)
            a1t_m(0, 0)
            a1t_m(0, 1)
            x1t_u(0)

            if debug:
                for u in range(16):
                    tcp = l2p.tile([128, K1], F32, tag="dbgcp", name="tcp")
                    nc.vector.tensor_copy(out=tcp[:], in_=t_bf[0][u][:])
                    nc.sync.dma_start(out=dbg["t"][u * 128:(u + 1) * 128, :],
                                      in_=tcp[:])

            # batch 1 level-1 with batch 0 level-2 stages 1..8 interleaved;
            # batch 0's remaining stages pair with batch 1's level-2 tail so
            # the tail's dependency stalls are filled with independent work
            stages0 = lvl2_stages(0)
            si = 0
            nunits = 29
            uidx = 0

            def sprinkle():
                nonlocal si, uidx
                uidx += 1
                while si < min(8, (uidx * 8 + nunits - 1) // nunits):
                    stages0[si]()
                    si += 1

            taccs1 = []
            for v in range(16):
                passA_v(1, v, taccs1)
                sprinkle()
            for u in range(NA, 16):
                passB_u(1, u)
                sprinkle()
            a1t_m(1, 0)
            sprinkle()
            a1t_m(1, 1)
            sprinkle()
            x1t_u(1)

            for st1 in lvl2_stages(1):
                if si < len(stages0):
                    stages0[si]()
                    si += 1
                st1()
            while si < len(stages0):
                stages0[si]()
                si += 1

            for b in range(BPC):
                nc.sync.dma_start(out=out[b:b + 1, :], in_=out_sbs[b][:])

    _legalize_multiwait(nc)
    return nc


# ---------------------------------------------------------------------------
# Host side
# ---------------------------------------------------------------------------
def _prep_inputs(inputs):
    inp = {k: np.asarray(v) for k, v in inputs.items()}
    sl1 = inp["slice_g1"].astype(np.int64)
    sl2 = inp["slice_g2"].astype(np.int64)
    b1 = inp["batch_g1"].astype(np.int64)
    b2 = inp["batch_g2"].astype(np.int64)
    n1 = np.diff(sl1)
    assert (n1 == N1P).all() and (np.diff(sl2) == N2P).all(), \
        "kernel hardcodes n1=1100/n2=900 per batch"
    pos1 = np.arange(inp["x_g1"].shape[0], dtype=np.int64) - sl1[b1]
    pos2 = (np.arange(inp["x_g2"].shape[0], dtype=np.int64) - sl2[b2]
            + n1[b2])

    # packed dense transposed features, bf16: cols 0:1152 hold g1 features at
    # node position, cols 1152:2176 hold g2 features at position-1024
    xtp = np.zeros((B, IN_DIM, XTC), np.float32)
    xg1t = inp["x_g1"].T
    xg2t = inp["x_g2"].T
    for b in range(B):
        r1 = slice(sl1[b], sl1[b + 1])
        xtp[b][:, pos1[r1]] = xg1t[:, r1]
        r2 = slice(sl2[b], sl2[b + 1])
        xtp[b][:, WTRIM + pos2[r2] - 1024] = xg2t[:, r2]
    xtp = xtp.astype(BFNP)

    # packed transposed dense adjacency [B, 128, TOTC]: partition = dst&127,
    # column = OFFV[dst>>7] + src; v-blocks 0..7 (dst<1024 => g1 dst =>
    # src < 1152) are width-trimmed. Per-partition rows are contiguous.
    e1, e2, eh = inp["edge_g1"], inp["edge_g2"], inp["edge_h"]
    eb = np.concatenate([b1[e1[0]], b2[e2[0]], b1[eh[0]]]).astype(np.int64)
    src = np.concatenate([pos1[e1[0]], pos2[e2[0]], pos1[eh[0]]])
    dst = np.concatenate([pos1[e1[1]], pos2[e2[1]], pos2[eh[1]]])
    vsl = dst >> 7
    offv = np.where(vsl < 8, vsl * WTRIM, 8 * WTRIM + (vsl - 8) * MN)
    if ADJ_FP8:
        adj_u = np.zeros((B, 128 * TOTC), np.uint8)
    else:
        adj_u = np.zeros((B, 128 * TOTC), np.uint16)
    adj_u[eb, (dst & 127) * TOTC + offv + src] = ADJ_ONE
    adj_np = adj_u.view(ADJ_NP).reshape(B, 128, TOTC)

    # fused projection weights [128, 538] bf16
    wallh = np.concatenate(
        [inp["W_pool_g1"], inp["W_emb_g1"], inp["W_pool_g2"], inp["W_emb_g2"]],
        axis=1).astype(np.float32).astype(BFNP)
    # packed level-2 weights [64, 472] f32
    w2h = np.zeros((HID, W2W), np.float32)
    for name, (rows, c0, c1) in _W2COLS.items():
        w2h[:rows, c0:c1] = inp[name]

    in_maps = []
    for c in range(NCORES):
        bs = slice(c * BPC, (c + 1) * BPC)
        in_maps.append(dict(
            xt=np.ascontiguousarray(xtp[bs]),
            adj=np.ascontiguousarray(adj_np[bs]),
            wall=wallh, w2=w2h,
        ))
    return in_maps


_NC_CACHE = {}


def run(inputs, debug=False, trace=False, tmpdir=None):
    key = bool(debug)
    if key not in _NC_CACHE:
        _NC_CACHE[key] = build_nc(debug=debug)
    nc = _NC_CACHE[key]
    in_maps = _prep_inputs(inputs)
    res = run_bass_kernel_spmd(nc, in_maps, list(range(NCORES)),
                               trace=trace, tmpdir=tmpdir)
    y = np.zeros((B, OUT), np.float32)
    for c in range(NCORES):
        o = res.results[c]["out"]       # [BPC, OUT]
        for b in range(BPC):
            y[c * BPC + b] = o[b]
    return y, res


def kernel(**inputs):
    y, _ = run(inputs)
    return y
